# revision 1
# baseline (speedup 1.0000x reference)
"""Trainium2 Bass kernel for the 5-stream dense transformer block
(nn_BWRPE_ater_19868518711491).

Strategy (8 NeuronCores, SPMD single program):
  - Token-sharded: core c owns token block n in [128c, 128(c+1)) for all 10
    (stream, batch) pairs.  LN1/adapter1/qkv (phase 1), proj/residuals/LN/
    MLP/adapter2 (phase 3) are fully local under this sharding.
  - Attention is sharded by (stream, batch, head) "units": 5*2*12 = 120
    units, 15 per core.  Phase 1 stages q/k/v head fragments into an
    AllToAll buffer; each core then owns the full sequence for its 15
    units.  A second AllToAll redistributes attention outputs back to
    token-sharded layout for phase 3.
  - All matmuls run in bf16 (fp32 PSUM accumulation); softmax uses
    exp(x*scale - B) with a fixed safe bound B (scores for this problem
    are ~N(0, 0.3)), normalized by a matmul-accumulated denominator
    (ones column appended to V).

The per-core program is identical (SPMD); all per-core variation enters
through the input shards.
"""

import contextlib

import numpy as np

import concourse.bacc as bacc
import concourse.bass as bass
import concourse.mybir as mybir
import concourse.tile as tile
from concourse.bass_utils import run_bass_kernel_spmd
from concourse.masks import make_identity

# problem shapes (hardcoded per harness contract)
S, B, N, C, H, AD, HID = 5, 2, 1024, 768, 12, 8, 3072
D = C // H              # 64 head dim
SCALE = D ** -0.5
NCORES = 8
NT = N // NCORES        # 128 tokens per core per (s,b)
NSB = S * B             # 10 (s,b) blocks per core
U = NSB * H             # 120 attention units
UPC = U // NCORES       # 15 units per core
P = 128
CC = C // P             # 6 contraction chunks of 128
EXP_BIAS = 6.0          # exp(score*scale - B); scores observed in [-2.6, 2.4]

F32 = mybir.dt.float32
BF16 = mybir.dt.bfloat16
AF = mybir.ActivationFunctionType

WEIGHT_NAMES = [
    "ln1_g", "ln1_b", "ln2_g", "ln2_b",
    "qkv_w", "qkv_b", "proj_w", "proj_b",
    "fc1_w", "fc1_b", "fc2_w", "fc2_b",
    "at_dw", "at_db", "at_mw", "at_mb", "at_uw", "at_ub",
    "a2_dw", "a2_db", "a2_mw", "a2_mb", "a2_uw", "a2_ub",
]


def _bcast_ap(param_ap: bass.AP, parts: int) -> bass.AP:
    """1-D DRAM tensor -> [parts, len] broadcast AP (partition step 0)."""
    (n,) = param_ap.shape
    return bass.AP(
        tensor=param_ap.tensor,
        offset=param_ap.offset,
        ap=[[0, parts], [1, n]],
    )


def build_program() -> bass.Bass:
    nc = bacc.Bacc(None, num_devices=NCORES)

    xs = nc.dram_tensor("xs", [NSB, NT, C], F32, kind="ExternalInput")
    w = {
        name: nc.dram_tensor(name, shape, F32, kind="ExternalInput")
        for name, shape in [
            ("ln1_g", [C]), ("ln1_b", [C]), ("ln2_g", [C]), ("ln2_b", [C]),
            ("qkv_w", [C, 3 * C]), ("qkv_b", [3 * C]),
            ("proj_w", [C, C]), ("proj_b", [C]),
            ("fc1_w", [C, HID]), ("fc1_b", [HID]),
            ("fc2_w", [HID, C]), ("fc2_b", [C]),
            ("at_dw", [C, AD]), ("at_db", [AD]),
            ("at_mw", [AD, AD]), ("at_mb", [AD]),
            ("at_uw", [AD, C]), ("at_ub", [C]),
            ("a2_dw", [C, AD]), ("a2_db", [AD]),
            ("a2_mw", [AD, AD]), ("a2_mb", [AD]),
            ("a2_uw", [AD, C]), ("a2_ub", [C]),
        ]
    }
    out = nc.dram_tensor("out", [NSB, NT, C], F32, kind="ExternalOutput")

    with tile.TileContext(nc) as tc:
        build_tile_kernel(tc, xs.ap(), {k: v.ap() for k, v in w.items()}, out.ap())
    nc.finalize()
    return nc


def build_tile_kernel(tc, xs, w, out):
    nc = tc.nc
    rg = [list(range(NCORES))]

    with contextlib.ExitStack() as ctx:
        sing = ctx.enter_context(tc.tile_pool(name="sing", bufs=1))
        s_fc = ctx.enter_context(tc.tile_pool(name="s_fc", bufs=1))
        dram = ctx.enter_context(tc.tile_pool(name="dram", bufs=1, space="DRAM"))

        # ---------------- constants ----------------
        ident = sing.tile([P, P], BF16)
        make_identity(nc, ident)
        eps_sb = sing.tile([P, 1], F32)
        nc.vector.memset(eps_sb, 1e-5)
        zero_sb = sing.tile([P, 1], F32)
        nc.vector.memset(zero_sb, 0.0)
        nexpb_sb = sing.tile([P, 1], F32)
        nc.vector.memset(nexpb_sb, -EXP_BIAS)
        ones1 = sing.tile([1, P], BF16)
        nc.vector.memset(ones1, 1.0)

        # ---------------- DRAM scratch ----------------
        # flat 8192-elem fragments per (unit, tensor): q/k stored transposed
        # [64 d, 128 n]; v natural [128 n, 64 d]; attention out transposed.
        # The first exchange is split in three unit segments so the later
        # collectives overlap with attention compute on earlier segments.
        SEGS = [(0, 8), (8, UPC)]
        a2a1_in0 = dram.tile([NCORES, 8, 3, NT * D], BF16)
        a2a1_out0 = dram.tile([NCORES, 8, 3, NT * D], BF16)
        a2a1_in1 = dram.tile([NCORES, UPC - 8, 3, NT * D], BF16)
        a2a1_out1 = dram.tile([NCORES, UPC - 8, 3, NT * D], BF16)
        a2a1_ins = [a2a1_in0, a2a1_in1]
        a2a1_outs = [a2a1_out0, a2a1_out1]

        def seg_of(lu):
            for si, (s0, s1) in enumerate(SEGS):
                if s0 <= lu < s1:
                    return si, lu - s0
            raise AssertionError
        # second exchange also split (units 0-7 / 8-14): the first chunk
        # transfers while phase 2 finishes the remaining units.
        a2a2_inA = dram.tile([NCORES, 8, NT * D], BF16)
        a2a2_outA = dram.tile([NCORES, 8, NT * D], BF16)
        a2a2_inB = dram.tile([NCORES, UPC - 8, NT * D], BF16)
        a2a2_outB = dram.tile([NCORES, UPC - 8, NT * D], BF16)
        a1_dram = dram.tile([NSB, NT, C], BF16)
        a2_dram = dram.tile([NSB, NT, C], BF16)
        zp_dram = dram.tile([NSB, NT, C], F32)

        # ---------------- helpers ----------------
        def load_cast(wst, pool, param, shape2d, name):
            """DRAM f32 [K, M] -> SBUF bf16 [pp, K//128, M] (K partition chunks).
            Issued on the ACT HWDGE queue to keep SP free for data DMAs."""
            k, m = shape2d
            kc = (k + P - 1) // P
            t = pool.tile([min(P, k), kc, m], BF16, name=name)
            for i in range(kc):
                rows = min(P, k - i * P)
                for lo in range(0, m, 1536):
                    hi = min(lo + 1536, m)
                    st = wst.tile([P, 1536], F32, tag="wstage")
                    nc.scalar.dma_start(
                        st[:rows, : hi - lo], param[i * P : i * P + rows, lo:hi]
                    )
                    nc.gpsimd.tensor_copy(t[:rows, i, lo:hi], st[:rows, : hi - lo])
            return t

        def load_bias_bcast(wst, pool, param, n, name):
            """1-D f32 [n] -> SBUF [128, n] bf16 broadcast tile."""
            t = pool.tile([P, n], BF16, name=name)
            for lo in range(0, n, 1536):
                hi = min(lo + 1536, n)
                st = wst.tile([P, 1536], F32, tag="wstage")
                nc.scalar.dma_start(st[:, : hi - lo], _bcast_ap(param[lo:hi], P))
                nc.gpsimd.tensor_copy(t[:, lo:hi], st[:, : hi - lo])
            return t

        def load_bias_row(wst, pool, param, n, name):
            """1-D f32 [n] -> SBUF [1, n] bf16 row (for matmul-bias)."""
            t = pool.tile([1, n], BF16, name=name)
            st = wst.tile([1, HID], F32, tag="rstage")
            nc.scalar.dma_start(st[:, :n], param[None, :])
            nc.gpsimd.tensor_copy(t[:], st[:, :n])
            return t

        def load_vec_col(pool, param, n, name):
            t = pool.tile([n, 1], F32, name=name)
            nc.scalar.dma_start(t[:], param[:, None])
            return t

        def layernorm(lnp, x_f32, gt, bt, out_dtype, name, inplace=False):
            """x [128, C] f32 -> (x-mu)/sqrt(var+eps)*g+b  [128, C] out_dtype."""
            stats = lnp.tile([P, 3, 6], F32, tag=f"{name}_stats")
            for i in range(3):
                nc.vector.bn_stats(stats[:, i, :], x_f32[:, i * 256 : (i + 1) * 256])
            mv = lnp.tile([P, 2], F32, tag=f"{name}_mv")
            nc.vector.bn_aggr(mv[:], stats[:])
            std = lnp.tile([P, 1], F32, tag=f"{name}_std")
            nc.scalar.activation(std[:], mv[:, 1:2], AF.Sqrt, bias=eps_sb, scale=1.0)
            rstd = lnp.tile([P, 1], F32, tag=f"{name}_rstd")
            nc.vector.reciprocal(rstd[:], std[:])
            if inplace:
                xn = x_f32
            else:
                xn_t = lnp.tile([P, C], F32, tag=f"{name}_xn", name=f"{name}_xn")
                xn = xn_t[:]
            nc.vector.tensor_scalar(
                xn, x_f32, mv[:, 0:1], rstd[:],
                mybir.AluOpType.subtract, mybir.AluOpType.mult,
            )
            o = lnp.tile([P, C], out_dtype, tag=f"{name}_o")
            nc.vector.tensor_mul(o[:], xn, gt[:])
            nc.vector.tensor_add(o[:], o[:], bt[:])
            return o

        def transpose_cc(pool, pst, src_bf16, ncc, name):
            """src [128, ncc*128] bf16 -> [128, ncc, 128] bf16 transposed chunks."""
            t = pool.tile([P, ncc, P], BF16, tag=name)
            for i in range(ncc):
                ps = pst.tile([P, P], BF16, tag="tr")
                nc.tensor.transpose(ps[:], src_bf16[:, i * P : (i + 1) * P], ident[:])
                nc.scalar.copy(t[:, i, :], ps[:])
            return t

        def adapter_tf(pool, psmm, pss, lnyT, wd, bd, wm, bm, wu, bu, name):
            """adapter on transposed input lnyT [128, 6, 128];
            returns natural [128, C] bf16 output (before residual)."""
            h1ps = pss.tile([AD, P], F32, tag="ad_ps")
            for cc in range(CC):
                nc.tensor.matmul(
                    h1ps[:], lhsT=wd[:, cc, :], rhs=lnyT[:, cc, :],
                    start=(cc == 0), stop=(cc == CC - 1),
                )
            h1 = pool.tile([AD, P], BF16, tag=f"{name}_h1")
            nc.scalar.activation(h1[:], h1ps[:], AF.Identity, bias=bd)
            h2ps = pss.tile([AD, P], F32, tag="ad_ps")
            nc.tensor.matmul(h2ps[:], lhsT=wm[:AD, 0, :], rhs=h1[:], start=True, stop=True)
            h2 = pool.tile([AD, P], BF16, tag=f"{name}_h2")
            nc.scalar.activation(h2[:], h2ps[:], AF.Identity, bias=bm)
            a = pool.tile([P, C], BF16, tag=f"{name}_a")
            for lo, hi in [(0, 512), (512, 768)]:
                ups = psmm.tile([P, 512], F32, tag="mm")
                nc.tensor.matmul(
                    ups[:, : hi - lo], lhsT=h2[:], rhs=wu[:AD, 0, lo:hi],
                    start=True, stop=True,
                )
                nc.vector.tensor_add(a[:, lo:hi], ups[:, : hi - lo], bu[:, lo:hi])
            return a

        # ---------------- load weights ----------------
        # small/common weights live for the whole kernel
        with tc.tile_pool(name="wst0", bufs=2) as wst:
            w_atd = load_cast(wst, sing, w["at_dw"], (C, AD), "w_atd")
            w_a2d = load_cast(wst, sing, w["a2_dw"], (C, AD), "w_a2d")
            w_atm = load_cast(wst, sing, w["at_mw"], (AD, AD), "w_atm")
            w_a2m = load_cast(wst, sing, w["a2_mw"], (AD, AD), "w_a2m")
            w_atu = load_cast(wst, sing, w["at_uw"], (AD, C), "w_atu")
            w_a2u = load_cast(wst, sing, w["a2_uw"], (AD, C), "w_a2u")
            b_proj = load_bias_bcast(wst, sing, w["proj_b"], C, "b_proj")
            b_fc2 = load_bias_bcast(wst, sing, w["fc2_b"], C, "b_fc2")
            b_atu = load_bias_bcast(wst, sing, w["at_ub"], C, "b_atu")
            b_a2u = load_bias_bcast(wst, sing, w["a2_ub"], C, "b_a2u")
            g_ln1 = load_bias_bcast(wst, sing, w["ln1_g"], C, "g_ln1")
            bb_ln1 = load_bias_bcast(wst, sing, w["ln1_b"], C, "bb_ln1")
            g_ln2 = load_bias_bcast(wst, sing, w["ln2_g"], C, "g_ln2")
            bb_ln2 = load_bias_bcast(wst, sing, w["ln2_b"], C, "bb_ln2")
            b_atd = load_vec_col(sing, w["at_db"], AD, "b_atd")
            b_atm = load_vec_col(sing, w["at_mb"], AD, "b_atm")
            b_a2d = load_vec_col(sing, w["a2_db"], AD, "b_a2d")
            b_a2m = load_vec_col(sing, w["a2_mb"], AD, "b_a2m")
            # fc1 bias as per-partition columns [128, 24] for the transposed MLP
            b_fc1c = sing.tile([P, HID // P], F32, name="b_fc1c")
            nc.scalar.dma_start(b_fc1c[:], w["fc1_b"].rearrange("(k p) -> p k", p=P))
            # phase-3 big weights (space reserved whole kernel; loads overlap p1/p2)
            w_proj = load_cast(wst, s_fc, w["proj_w"], (C, C), "w_proj")
            w_fc1 = load_cast(wst, s_fc, w["fc1_w"], (C, HID), "w_fc1")
            w_fc2 = load_cast(wst, s_fc, w["fc2_w"], (HID, C), "w_fc2")

        # ================= phase 1 =================
        with contextlib.ExitStack() as p1ctx:
            s_qkv = p1ctx.enter_context(tc.tile_pool(name="s_qkv", bufs=1))
            with tc.tile_pool(name="wst1", bufs=2) as wst:
                w_qkv = load_cast(wst, s_qkv, w["qkv_w"], (C, 3 * C), "w_qkv")
                b_qkvr = load_bias_row(wst, s_qkv, w["qkv_b"], 3 * C, "b_qkvr")
            p1 = p1ctx.enter_context(tc.tile_pool(name="p1", bufs=3))
            ps_mm = p1ctx.enter_context(tc.tile_pool(name="ps_mm1", bufs=3, space="PSUM"))
            ps_tr = p1ctx.enter_context(tc.tile_pool(name="ps_tr1", bufs=3, space="PSUM"))
            ps_s = p1ctx.enter_context(tc.tile_pool(name="ps_s1", bufs=2, space="PSUM"))

            for sb in range(NSB):
                x_t = p1.tile([P, C], F32, tag="x1")
                nc.sync.dma_start(x_t[:], xs[sb])
                n1 = layernorm(p1, x_t[:], g_ln1, bb_ln1, BF16, "ln1", inplace=True)
                n1T = transpose_cc(p1, ps_tr, n1, CC, "n1T")

                # qkv = n1 @ qkv_w + b  (natural layout [128, 2304] bf16)
                qkv_bf = p1.tile([P, 3 * C], BF16, tag="qkv_bf")
                for j in range(5):  # col chunks of <=512
                    lo, hi = j * 512, min((j + 1) * 512, 3 * C)
                    ps = ps_mm.tile([P, 512], F32, tag="mm")
                    for cc in range(CC):
                        nc.tensor.matmul(
                            ps[:, : hi - lo], lhsT=n1T[:, cc, :],
                            rhs=w_qkv[:, cc, lo:hi],
                            start=(cc == 0), stop=False,
                        )
                    nc.tensor.matmul(
                        ps[:, : hi - lo], lhsT=ones1[:], rhs=b_qkvr[:, lo:hi],
                        start=False, stop=True,
                    )
                    nc.vector.tensor_copy(qkv_bf[:, lo:hi], ps[:, : hi - lo])

                # adapter 1 (input n1T) -> DRAM
                a1 = adapter_tf(p1, ps_mm, ps_s, n1T,
                                w_atd, b_atd, w_atm, b_atm, w_atu, b_atu, "a1")
                nc.sync.dma_start(a1_dram[sb], a1[:])

                # stage q/k TRANSPOSED fragments: PE-transpose the 6 chunks
                # (2 heads each), then one strided DMA per (dst-group, parity).
                for t, base in ((0, 0), (1, C)):
                    qkT = p1.tile([P, CC, P], BF16, tag="qkT")
                    for cc_ in range(CC):
                        pst = ps_tr.tile([P, P], BF16, tag="tr")
                        nc.tensor.transpose(
                            pst[:], qkv_bf[:, base + cc_ * P : base + (cc_ + 1) * P],
                            ident[:],
                        )
                        nc.vector.tensor_copy(qkT[:, cc_, :], pst[:])
                    groups = {}
                    for h in range(H):
                        u = sb * H + h
                        key = (u // UPC, seg_of(u % UPC)[0], h % 2)
                        groups.setdefault(key, []).append(h)
                    for (dst, si, par), hs in groups.items():
                        nhh = len(hs)
                        src = qkT[par * D : (par + 1) * D,
                                  hs[0] // 2 : hs[0] // 2 + nhh, :]
                        l0 = seg_of((sb * H + hs[0]) % UPC)[1]
                        dst_ap = a2a1_ins[si][dst, l0 : l0 + 2 * nhh - 1 : 2, t]
                        dst_ap = dst_ap.rearrange("h (d n) -> d h n", n=P)
                        nc.sync.dma_start(dst_ap, src)
                # v fragments natural, one DMA per (dst-group, segment)
                groups = {}
                for h in range(H):
                    u = sb * H + h
                    groups.setdefault((u // UPC, seg_of(u % UPC)[0]), []).append(h)
                for (dst, si), hs in groups.items():
                    h0, nh = hs[0], len(hs)
                    l0 = seg_of((sb * H + h0) % UPC)[1]
                    src = qkv_bf[:, 2 * C + h0 * D : 2 * C + (h0 + nh) * D].rearrange(
                        "p (h d) -> p h d", d=D
                    )
                    dst_ap = a2a1_ins[si][dst, l0 : l0 + nh, 2].rearrange(
                        "h (n d) -> n h d", d=D
                    )
                    nc.sync.dma_start(dst_ap, src)

        for si in range(len(SEGS)):
            nc.gpsimd.collective_compute(
                "AllToAll", mybir.AluOpType.bypass, replica_groups=rg,
                ins=[a2a1_ins[si][:].opt()], outs=[a2a1_outs[si][:].opt()],
            )

        # ================= phase 2: attention units =================
        with contextlib.ExitStack() as p2ctx:
            p2 = p2ctx.enter_context(tc.tile_pool(name="p2", bufs=3))
            p2e = p2ctx.enter_context(tc.tile_pool(name="p2e", bufs=2))
            ps_sc = p2ctx.enter_context(tc.tile_pool(name="ps_sc", bufs=2, space="PSUM"))
            ps_o = p2ctx.enter_context(tc.tile_pool(name="ps_o", bufs=2, space="PSUM"))
            ps_t2 = p2ctx.enter_context(tc.tile_pool(name="ps_t2", bufs=2, space="PSUM"))

            for lu in range(UPC):
                si, l = seg_of(lu)
                buf = a2a1_outs[si]
                qt = p2.tile([D, N], BF16, tag="qt")
                kt = p2.tile([D, N], BF16, tag="kt")
                nc.sync.dma_start(
                    qt[:].rearrange("d (j n) -> d j n", n=NT),
                    buf[:, l, 0].rearrange("j (d n) -> d j n", n=NT),
                )
                nc.sync.dma_start(
                    kt[:].rearrange("d (j n) -> d j n", n=NT),
                    buf[:, l, 1].rearrange("j (d n) -> d j n", n=NT),
                )
                v_sb = p2.tile([P, NCORES, D + 1], BF16, tag="v_sb")
                nc.vector.memset(v_sb[:, :, D : D + 1], 1.0)
                nc.sync.dma_start(
                    v_sb[:, :, :D],
                    buf[:, l, 2].rearrange("j (n d) -> n j d", d=D),
                )

                expT = p2e.tile([P, NCORES, N], BF16, tag="expT")
                for mt in range(NCORES):
                    ps = ps_sc.tile([P, N], F32, tag="scoresT")
                    for half in range(2):
                        nc.tensor.matmul(
                            ps[:, half * 512 : (half + 1) * 512],
                            lhsT=kt[:, mt * NT : (mt + 1) * NT],
                            rhs=qt[:, half * 512 : (half + 1) * 512],
                            start=True, stop=True,
                        )
                    nc.scalar.activation(
                        expT[:, mt, :], ps[:], AF.Exp, bias=nexpb_sb, scale=SCALE,
                    )

                for nqb in range(NCORES):
                    ops = ps_o.tile([P, D + 8], F32, tag="o_ps")
                    for mt in range(NCORES):
                        nc.tensor.matmul(
                            ops[:, : D + 1],
                            lhsT=expT[:, mt, nqb * NT : (nqb + 1) * NT],
                            rhs=v_sb[:, mt, :],
                            start=(mt == 0), stop=(mt == NCORES - 1),
                        )
                    rden = p2.tile([P, 1], F32, tag="rden")
                    nc.vector.reciprocal(rden[:], ops[:, D : D + 1])
                    o_bf = p2.tile([P, D], BF16, tag="o_bf")
                    nc.vector.tensor_scalar_mul(o_bf[:], ops[:, :D], rden[:])
                    # write O transposed [64 d, 128 n] so phase 3 can bulk-load
                    pst = ps_t2.tile([D, P], BF16, tag="tr2")
                    nc.tensor.transpose(pst[:], o_bf[:], ident[:])
                    oT_sb = p2.tile([D, P], BF16, tag="oT_sb")
                    nc.vector.tensor_copy(oT_sb[:], pst[:])
                    buf2 = a2a2_inA if lu < 8 else a2a2_inB
                    l2 = lu if lu < 8 else lu - 8
                    nc.sync.dma_start(
                        buf2[nqb, l2].rearrange("(d n) -> d n", n=P), oT_sb[:]
                    )

        nc.gpsimd.collective_compute(
            "AllToAll", mybir.AluOpType.bypass, replica_groups=rg,
            ins=[a2a2_inA[:].opt()], outs=[a2a2_outA[:].opt()],
        )
        nc.gpsimd.collective_compute(
            "AllToAll", mybir.AluOpType.bypass, replica_groups=rg,
            ins=[a2a2_inB[:].opt()], outs=[a2a2_outB[:].opt()],
        )

        def a2a2_frag(u):
            lu = u % UPC
            if lu < 8:
                return a2a2_outA[u // UPC, lu]
            return a2a2_outB[u // UPC, lu - 8]

        # ================= phase 3 =================
        with contextlib.ExitStack() as p3ctx:
            p3 = p3ctx.enter_context(tc.tile_pool(name="p3", bufs=3))
            p3ln = p3ctx.enter_context(tc.tile_pool(name="p3ln", bufs=3))
            p3g = p3ctx.enter_context(tc.tile_pool(name="p3g", bufs=2))
            ps_mm = p3ctx.enter_context(tc.tile_pool(name="ps_mm3", bufs=4, space="PSUM"))
            ps_tr = p3ctx.enter_context(tc.tile_pool(name="ps_tr3", bufs=2, space="PSUM"))
            ps_s = p3ctx.enter_context(tc.tile_pool(name="ps_s3", bufs=2, space="PSUM"))

            for sb in range(NSB):
                oT = p3.tile([P, CC, P], BF16, tag="oT")
                for cc in range(CC):
                    u = sb * H + 2 * cc
                    same_core = u // UPC == (u + 1) // UPC
                    same_buf = (u % UPC < 8) == ((u + 1) % UPC < 8)
                    if same_core and same_buf:
                        # adjacent fragments in the same buffer: one DMA
                        buf2 = a2a2_outA if u % UPC < 8 else a2a2_outB
                        l2 = u % UPC if u % UPC < 8 else u % UPC - 8
                        nc.sync.dma_start(
                            oT[:, cc, :],
                            buf2[u // UPC, l2 : l2 + 2].rearrange(
                                "h (d n) -> (h d) n", n=P
                            ),
                        )
                    else:
                        for i, uu in ((0, u), (1, u + 1)):
                            nc.sync.dma_start(
                                oT[i * D : (i + 1) * D, cc, :],
                                a2a2_frag(uu).rearrange("(d n) -> d n", n=P),
                            )
                # y = x + (o @ proj_w + b) + a1 neighbors
                y = p3.tile([P, C], F32, tag="y")
                x_t = p3.tile([P, C], F32, tag="xio")
                nc.sync.dma_start(x_t[:], xs[sb])
                for lo, hi in [(0, 512), (512, 768)]:
                    prps = ps_mm.tile([P, 512], F32, tag="mm")
                    for cc in range(CC):
                        nc.tensor.matmul(
                            prps[:, : hi - lo], lhsT=oT[:, cc, :],
                            rhs=w_proj[:, cc, lo:hi],
                            start=(cc == 0), stop=(cc == CC - 1),
                        )
                    nc.vector.tensor_add(y[:, lo:hi], prps[:, : hi - lo], b_proj[:, lo:hi])
                nc.vector.tensor_add(y[:], y[:], x_t[:])
                for nb in (sb - 2, sb + 2):
                    if 0 <= nb < NSB:
                        a1n = p3.tile([P, C], BF16, tag="nres")
                        nc.sync.dma_start(a1n[:], a1_dram[nb])
                        nc.vector.tensor_add(y[:], y[:], a1n[:])

                lny1 = layernorm(p3ln, y[:], g_ln1, bb_ln1, BF16, "lny")
                lny2 = layernorm(p3ln, y[:], g_ln2, bb_ln2, BF16, "lny")
                lny1T = transpose_cc(p3ln, ps_tr, lny1, CC, "lnyT")
                lny2T = transpose_cc(p3ln, ps_tr, lny2, CC, "lnyT")

                a2 = adapter_tf(p3, ps_mm, ps_s, lny1T,
                                w_a2d, b_a2d, w_a2m, b_a2m, w_a2u, b_a2u, "a2")
                nc.sync.dma_start(a2_dram[sb], a2[:])

                # MLP: hT computed directly in transposed form (hid on
                # partitions); gelu + per-partition bias fused in the evac.
                ghT = p3g.tile([P, HID // P, P], BF16, tag="ghT")
                for kk in range(HID // P):
                    ps = ps_mm.tile([P, 512], F32, tag="mm")
                    for cc in range(CC):
                        nc.tensor.matmul(
                            ps[:, :P], lhsT=w_fc1[:, cc, kk * P : (kk + 1) * P],
                            rhs=lny2T[:, cc, :],
                            start=(cc == 0), stop=(cc == CC - 1),
                        )
                    nc.scalar.activation(
                        ghT[:, kk, :], ps[:, :P], AF.Gelu,
                        bias=b_fc1c[:, kk : kk + 1],
                    )

                zp = p3.tile([P, C], F32, tag="zp")
                for lo, hi in [(0, 512), (512, 768)]:
                    m2 = ps_mm.tile([P, 512], F32, tag="mm")
                    for kk in range(HID // P):
                        nc.tensor.matmul(
                            m2[:, : hi - lo], lhsT=ghT[:, kk, :],
                            rhs=w_fc2[:, kk, lo:hi],
                            start=(kk == 0), stop=(kk == HID // P - 1),
                        )
                    nc.vector.tensor_add(zp[:, lo:hi], m2[:, : hi - lo], b_fc2[:, lo:hi])
                nc.vector.tensor_add(zp[:], zp[:], y[:])
                nc.sync.dma_start(zp_dram[sb], zp[:])

            # ---- cross-stream a2 residuals ----
            for sb in range(NSB):
                zt = p3.tile([P, C], F32, tag="xio")
                nc.sync.dma_start(zt[:], zp_dram[sb])
                for nb in (sb - 2, sb + 2):
                    if 0 <= nb < NSB:
                        a2n = p3.tile([P, C], BF16, tag="nres")
                        nc.sync.dma_start(a2n[:], a2_dram[nb])
                        nc.vector.tensor_add(zt[:], zt[:], a2n[:])
                nc.sync.dma_start(out[sb], zt[:])


_CACHED_NC = None


def _get_program():
    global _CACHED_NC
    if _CACHED_NC is None:
        _CACHED_NC = build_program()
    return _CACHED_NC


def kernel(**inputs) -> np.ndarray:
    nc = _get_program()

    xs_full = np.stack(
        [np.asarray(inputs[f"x{i}"], dtype=np.float32) for i in range(S)]
    )
    weights = {k: np.ascontiguousarray(np.asarray(inputs[k], dtype=np.float32))
               for k in WEIGHT_NAMES}

    in_maps = []
    for c in range(NCORES):
        shard = np.ascontiguousarray(
            xs_full[:, :, c * NT : (c + 1) * NT, :].reshape(NSB, NT, C)
        )
        m = {"xs": shard}
        m.update(weights)
        in_maps.append(m)

    res = run_bass_kernel_spmd(nc, in_maps, core_ids=list(range(NCORES)))
    z = np.empty((S, B, N, C), dtype=np.float32)
    for c in range(NCORES):
        z[:, :, c * NT : (c + 1) * NT, :] = res.results[c]["out"].reshape(S, B, NT, C)
    return z

